# revision 2
# baseline (speedup 1.0000x reference)
"""Trainium2 Bass kernel for the tree-GNN message-passing model.

Model (per sample): 9 splits of a binary tree, each step:
  g = glob_post(sum_nodes(glob_pre(x)))            # DeepSets global
  children = proj(concat(leaves, g))               # split leaves
  m_v = msg(concat(x_v, g))  for all current nodes
  aggr_u = sum_{v strict ancestor of u} m_v        # == tree prefix-sum of m
  x = update(concat(x, aggr, g))                   # all nodes

Key device mapping:
  - feature-major layout: features on SBUF partitions, (node*32+sample) on free dim
  - 32 samples per NeuronCore (8 cores, pure data parallel)
  - all matmuls float32r (fp32 with 11-bit mantissa rounding, full PE rate at N>=256)
  - ancestor scatter-add replaced by per-level prefix: s_l = m_l + spread(s_{l-1});
    aggr for level l read directly as a broadcast AP over s_{l-1} (never materialized)
  - x lives in DRAM ping-pong buffers (f32r bits), m lives in SBUF
"""
import os
import numpy as np
from contextlib import ExitStack

import concourse.bass as bass
import concourse.mybir as mybir
import concourse.tile as tile
from concourse import bacc
from concourse.bass_utils import run_bass_kernel_spmd

F32 = mybir.dt.float32
F32R = mybir.dt.float32r
RELU = mybir.ActivationFunctionType.Relu
ADD = mybir.AluOpType.add
MAX = mybir.AluOpType.max

NCORES = 8
B = 256
BPC = B // NCORES          # samples per core = 32
F = 64                     # features
G = 64                     # global features
NS = int(os.environ.get("KERNEL_NS", "9"))   # splits (dev knob)
OFF = [0, 1, 3, 7, 15, 31, 63, 127, 255, 511, 1023]   # level offsets in groups
NODES = OFF[NS + 1]        # 1023
XCOLS = NODES * BPC        # 32736
MCOLS = OFF[NS] * BPC      # 16352 (sources are levels 0..8)
CH = 1024                  # chunk width in columns (multiple of 512)
MMN = 512                  # max moving free dim per matmul

_cache = {"nc": None}


def _mm_split(nc, psum, lhsT, rhs_fn, ncols, start, stop):
    """Emit matmuls in <=MMN column pieces. rhs_fn(lo, n) -> rhs AP."""
    lo = 0
    while lo < ncols:
        n = min(MMN, ncols - lo)
        nc.tensor.matmul(
            psum[:, lo:lo + n], lhsT, rhs_fn(lo, n),
            start=start, stop=stop,
        )
        lo += n


def _chunks(lo_g, hi_g, ch_groups):
    """Yield (g0, ng) chunks of the group range [lo_g, hi_g)."""
    g = lo_g
    while g < hi_g:
        ng = min(ch_groups, hi_g - g)
        yield g, ng
        g += ng


def _level_chunks(lvl_lo, lvl_hi, ch_groups):
    """Level-aligned chunks: (level, g0, ng) covering levels [lvl_lo, lvl_hi]."""
    for l in range(lvl_lo, lvl_hi + 1):
        for g0, ng in _chunks(OFF[l], OFF[l + 1], ch_groups):
            yield l, g0, ng


def build_nc():
    nc = bacc.Bacc("TRN2", target_bir_lowering=False, debug=False,
                   num_devices=NCORES)

    # ---- I/O -------------------------------------------------------------
    xin = nc.dram_tensor("xin", [F, BPC], F32R, kind="ExternalInput").ap()
    wnames = {
        "gp0": [F, F], "gp1": [F, F], "gq0": [G, G], "gq1": [G, G],
        "pj0x": [F, 2 * F], "pj0g": [G, 2 * F], "pj1": [2 * F, 2 * F],
        "pj2": [2 * F, 2 * F],
        "ms0x": [F, F], "ms0g": [G, F], "ms1": [F, F], "ms2": [F, F],
        "up0x": [F, 2 * F + G], "up0a": [F, 2 * F + G], "up0g": [G, 2 * F + G],
        "up1a": [2 * F, F], "up1b": [G, F], "up2": [F, F],
    }
    bnames = {
        "bgp0": F, "bgp1": F, "bgq0": G, "bgq1": G,
        "bpj0": 2 * F, "bpj1": 2 * F, "bpj2a": F, "bpj2b": F,
        "bms0": F, "bms1": F, "bms2": F,
        "bup0a": 2 * F, "bup0b": G, "bup1": F, "bup2": F,
    }
    wdram = {k: nc.dram_tensor(k, v, F32R, kind="ExternalInput").ap()
             for k, v in wnames.items()}
    bdram = {k: nc.dram_tensor(k, [v, 1], F32, kind="ExternalInput").ap()
             for k, v in bnames.items()}
    xout = nc.dram_tensor("xout", [F, XCOLS], F32, kind="ExternalOutput").ap()
    # x ping-pong scratch in DRAM (f32r bits)
    xbuf = [nc.dram_tensor(f"xsc{i}", [F, XCOLS], F32R, kind="Internal").ap()
            for i in range(2)]

    chg = CH // BPC   # chunk width in groups (32)

    with tile.TileContext(nc) as tc:
        with ExitStack() as ctx:
            wpool = ctx.enter_context(tc.tile_pool(name="wpool", bufs=1))
            mpool = ctx.enter_context(tc.tile_pool(name="mpool", bufs=1))
            xch = ctx.enter_context(tc.tile_pool(name="xch", bufs=6))
            hp = ctx.enter_context(tc.tile_pool(name="hp", bufs=4))
            oup = ctx.enter_context(tc.tile_pool(name="oup", bufs=2))
            gp = ctx.enter_context(tc.tile_pool(name="gp", bufs=2))
            pa = ctx.enter_context(tc.tile_pool(name="pa", bufs=2, space="PSUM"))
            pb = ctx.enter_context(tc.tile_pool(name="pb", bufs=2, space="PSUM"))

            # ---- load weights / biases ----------------------------------
            W = {}
            for k, shp in wnames.items():
                t = wpool.tile(shp, F32R, name=f"w_{k}")
                nc.sync.dma_start(t, wdram[k])
                W[k] = t
            Bb = {}
            for k, n in bnames.items():
                t = wpool.tile([n, 1], F32, name=f"b_{k}")
                nc.sync.dma_start(t, bdram[k])
                Bb[k] = t
            zeros = wpool.tile([128, CH], F32, name="zeros")
            nc.vector.memset(zeros, 0.0)

            # msg outputs / prefix sums (levels 0..8), SBUF resident
            m_sb = mpool.tile([F, MCOLS], F32R, name="m_sb")

            # ---- initial x ----------------------------------------------
            x0 = oup.tile([F, BPC], F32R, name="x0")
            nc.sync.dma_start(x0, xin)
            nc.sync.dma_start(xbuf[0][:, 0:BPC], x0)

            for t in range(NS):
                n_now_g = OFF[t + 1]
                n_next_g = OFF[t + 2]
                xc = xbuf[t % 2]
                xn = xbuf[(t + 1) % 2]
                is_last = t == NS - 1

                # ===== global aggregation: g ==============================
                pacc = gp.tile([G, BPC], F32, name="pacc", tag="pacc")
                nc.vector.memset(pacc, 0.0)
                for g0, ng in _chunks(0, n_now_g, chg):
                    cols = ng * BPC
                    xt = xch.tile([F, CH], F32R, name="xt_g", tag="xt")
                    nc.sync.dma_start(xt[:, 0:cols],
                                      xc[:, g0 * BPC:(g0 + ng) * BPC])
                    p1 = pb.tile([F, CH], F32, name="pg1", tag="pb")
                    _mm_split(nc, p1, W["gp0"],
                              lambda lo, n: xt[:, lo:lo + n], cols, True, True)
                    h1 = hp.tile([F, CH], F32R, name="hg1", tag="h64")
                    nc.scalar.activation(h1[:, 0:cols], p1[:, 0:cols], RELU,
                                         bias=Bb["bgp0"])
                    p2 = pb.tile([F, CH], F32, name="pg2", tag="pb")
                    _mm_split(nc, p2, W["gp1"],
                              lambda lo, n: h1[:, lo:lo + n], cols, True, True)
                    h2 = hp.tile([F, CH], F32, name="hg2", tag="h64")
                    nc.vector.scalar_tensor_tensor(
                        h2[:, 0:cols], p2[:, 0:cols], Bb["bgp1"],
                        zeros[0:F, 0:cols], ADD, MAX)
                    # pool: sum over groups -> [G, BPC]
                    red = gp.tile([G, BPC, 1], F32, name="red", tag="red")
                    v = h2[:, 0:cols].rearrange("p (g s) -> p s g", s=BPC)
                    nc.vector.tensor_reduce(red, v, mybir.AxisListType.X, ADD)
                    nc.gpsimd.tensor_tensor(pacc, pacc, red.squeeze(2), ADD)

                pooled = gp.tile([G, BPC], F32R, name="pooled", tag="pooled")
                nc.vector.tensor_copy(pooled, pacc)
                pq1 = pb.tile([G, BPC], F32, name="pq1", tag="pb")
                nc.tensor.matmul(pq1, W["gq0"], pooled, start=True, stop=True)
                hq1 = gp.tile([G, BPC], F32R, name="hq1", tag="hq1")
                nc.scalar.activation(hq1, pq1, RELU, bias=Bb["bgq0"])
                pq2 = pb.tile([G, BPC], F32, name="pq2", tag="pb")
                nc.tensor.matmul(pq2, W["gq1"], hq1, start=True, stop=True)
                gt = gp.tile([G, BPC], F32R, name="gt", tag="gt")
                nc.scalar.activation(gt, pq2, RELU, bias=Bb["bgq1"])
                # broadcast g across groups: [G, MMN]
                gb = gp.tile([G, MMN], F32R, name="gb", tag="gb")
                nc.gpsimd.tensor_copy(
                    gb.rearrange("p (g s) -> p g s", s=BPC),
                    gt.unsqueeze(1).broadcast_to([G, MMN // BPC, BPC]))

                def gb_rhs(lo, n):
                    # g_bcast slice; lo is a multiple of 32 -> phase-aligned
                    return gb[:, 0:n]

                # ===== proj: split leaves into children ===================
                for g0, ng in _chunks(OFF[t], n_now_g, chg):
                    cols = ng * BPC
                    xt = xch.tile([F, CH], F32R, name="xt_p", tag="xt")
                    nc.sync.dma_start(xt[:, 0:cols],
                                      xc[:, g0 * BPC:(g0 + ng) * BPC])
                    p1 = pa.tile([2 * F, CH], F32, name="pp1", tag="pa")
                    _mm_split(nc, p1, W["pj0x"],
                              lambda lo, n: xt[:, lo:lo + n], cols, True, False)
                    _mm_split(nc, p1, W["pj0g"], gb_rhs, cols, False, True)
                    h1 = hp.tile([2 * F, CH], F32R, name="hp1", tag="h128")
                    nc.scalar.activation(h1[:, 0:cols], p1[:, 0:cols], RELU,
                                         bias=Bb["bpj0"])
                    p2 = pa.tile([2 * F, CH], F32, name="pp2", tag="pa")
                    _mm_split(nc, p2, W["pj1"],
                              lambda lo, n: h1[:, lo:lo + n], cols, True, True)
                    h2 = hp.tile([2 * F, CH], F32R, name="hp2", tag="h128")
                    nc.vector.scalar_tensor_tensor(
                        h2[:, 0:cols], p2[:, 0:cols], Bb["bpj1"],
                        zeros[0:2 * F, 0:cols], ADD, MAX)
                    p3a = pb.tile([F, CH], F32, name="pp3a", tag="pb")
                    _mm_split(nc, p3a, W["pj2"][:, 0:F],
                              lambda lo, n: h2[:, lo:lo + n], cols, True, True)
                    p3b = pb.tile([F, CH], F32, name="pp3b", tag="pb")
                    _mm_split(nc, p3b, W["pj2"][:, F:2 * F],
                              lambda lo, n: h2[:, lo:lo + n], cols, True, True)
                    # children staging, laid out exactly as x children range
                    chld = oup.tile([F, 2 * CH], F32R, name="chld", tag="chld")
                    cv = chld[:, 0:2 * cols].rearrange(
                        "p (i c s) -> p i c s", c=2, s=BPC)
                    nc.scalar.activation(
                        cv[:, :, 0, :],
                        p3a[:, 0:cols].rearrange("p (i s) -> p i s", s=BPC),
                        RELU, bias=Bb["bpj2a"])
                    nc.scalar.activation(
                        cv[:, :, 1, :],
                        p3b[:, 0:cols].rearrange("p (i s) -> p i s", s=BPC),
                        RELU, bias=Bb["bpj2b"])
                    cg0 = n_now_g + 2 * (g0 - OFF[t])
                    nc.sync.dma_start(
                        xc[:, cg0 * BPC:(cg0 + 2 * ng) * BPC],
                        chld[:, 0:2 * cols])

                # ===== msg MLP on all source nodes ========================
                for l, g0, ng in _level_chunks(0, t, chg):
                    cols = ng * BPC
                    xt = xch.tile([F, CH], F32R, name="xt_m", tag="xt")
                    nc.sync.dma_start(xt[:, 0:cols],
                                      xc[:, g0 * BPC:(g0 + ng) * BPC])
                    p1 = pb.tile([F, CH], F32, name="pm1", tag="pb")
                    _mm_split(nc, p1, W["ms0x"],
                              lambda lo, n: xt[:, lo:lo + n], cols, True, False)
                    _mm_split(nc, p1, W["ms0g"], gb_rhs, cols, False, True)
                    h1 = hp.tile([F, CH], F32R, name="hm1", tag="h64")
                    nc.scalar.activation(h1[:, 0:cols], p1[:, 0:cols], RELU,
                                         bias=Bb["bms0"])
                    p2 = pb.tile([F, CH], F32, name="pm2", tag="pb")
                    _mm_split(nc, p2, W["ms1"],
                              lambda lo, n: h1[:, lo:lo + n], cols, True, True)
                    h2 = hp.tile([F, CH], F32R, name="hm2", tag="h64")
                    nc.vector.scalar_tensor_tensor(
                        h2[:, 0:cols], p2[:, 0:cols], Bb["bms1"],
                        zeros[0:F, 0:cols], ADD, MAX)
                    p3 = pb.tile([F, CH], F32, name="pm3", tag="pb")
                    _mm_split(nc, p3, W["ms2"],
                              lambda lo, n: h2[:, lo:lo + n], cols, True, True)
                    nc.scalar.activation(
                        m_sb[:, g0 * BPC:(g0 + ng) * BPC],
                        p3[:, 0:cols], RELU, bias=Bb["bms2"])

                # ===== prefix sums: s_l = m_l + spread(s_{l-1}) ===========
                for l in range(1, t + 1):
                    par = m_sb[:, OFF[l - 1] * BPC:OFF[l] * BPC]
                    n_par = OFF[l] - OFF[l - 1]
                    spread = par.rearrange("p (i s) -> p i s", s=BPC) \
                        .unsqueeze(2).broadcast_to([F, n_par, 2, BPC])
                    cur = m_sb[:, OFF[l] * BPC:OFF[l + 1] * BPC]
                    nc.gpsimd.tensor_tensor(cur, cur, spread, ADD)

                # ===== update MLP on all nodes ============================
                def anc_rhs_fn(l, g0):
                    # broadcast AP over s_{l-1} for the chunk at groups
                    # [g0, g0+ng) of level l; column piece (lo, n)
                    def fn(lo, n):
                        # parent group of column c = OFF[l-1] + (g0-OFF[l])//2 + c//64
                        p0 = OFF[l - 1] + (g0 - OFF[l]) // 2
                        np_ = n // (2 * BPC)
                        base = p0 + lo // (2 * BPC)
                        par = m_sb[:, base * BPC:(base + np_) * BPC]
                        return par.rearrange("p (i s) -> p i s", s=BPC) \
                            .unsqueeze(2).broadcast_to([F, np_, 2, BPC])
                    return fn

                for l, g0, ng in _level_chunks(0, t + 1, chg):
                    cols = ng * BPC
                    xt = xch.tile([F, CH], F32R, name="xt_u", tag="xt")
                    nc.sync.dma_start(xt[:, 0:cols],
                                      xc[:, g0 * BPC:(g0 + ng) * BPC])
                    afn = anc_rhs_fn(l, g0) if l > 0 else None
                    p1a = pa.tile([2 * F, CH], F32, name="pu1a", tag="pa")
                    p1b = pb.tile([G, CH], F32, name="pu1b", tag="pb")
                    for psum, csel in ((p1a, slice(0, 2 * F)),
                                       (p1b, slice(2 * F, 2 * F + G))):
                        _mm_split(nc, psum, W["up0x"][:, csel],
                                  lambda lo, n: xt[:, lo:lo + n],
                                  cols, True, False)
                        if afn is not None:
                            _mm_split(nc, psum, W["up0a"][:, csel], afn,
                                      cols, False, False)
                        _mm_split(nc, psum, W["up0g"][:, csel], gb_rhs,
                                  cols, False, True)
                    h1a = hp.tile([2 * F, CH], F32R, name="hu1a", tag="h128")
                    nc.scalar.activation(h1a[:, 0:cols], p1a[:, 0:cols], RELU,
                                         bias=Bb["bup0a"])
                    h1b = hp.tile([G, CH], F32R, name="hu1b", tag="h64")
                    nc.vector.scalar_tensor_tensor(
                        h1b[:, 0:cols], p1b[:, 0:cols], Bb["bup0b"],
                        zeros[0:G, 0:cols], ADD, MAX)
                    p2 = pb.tile([F, CH], F32, name="pu2", tag="pb")
                    _mm_split(nc, p2, W["up1a"],
                              lambda lo, n: h1a[:, lo:lo + n], cols, True, False)
                    _mm_split(nc, p2, W["up1b"],
                              lambda lo, n: h1b[:, lo:lo + n], cols, False, True)
                    h2 = hp.tile([F, CH], F32R, name="hu2", tag="h64")
                    nc.vector.scalar_tensor_tensor(
                        h2[:, 0:cols], p2[:, 0:cols], Bb["bup1"],
                        zeros[0:F, 0:cols], ADD, MAX)
                    p3 = pb.tile([F, CH], F32, name="pu3", tag="pb")
                    _mm_split(nc, p3, W["up2"],
                              lambda lo, n: h2[:, lo:lo + n], cols, True, True)
                    ou = oup.tile([F, CH], F32R, name="ou", tag="ou")
                    nc.scalar.activation(ou[:, 0:cols], p3[:, 0:cols], RELU,
                                         bias=Bb["bup2"])
                    dst = xout if is_last else xn
                    src = ou[:, 0:cols].bitcast(F32) if is_last else ou[:, 0:cols]
                    nc.sync.dma_start(
                        dst[:, g0 * BPC:(g0 + ng) * BPC], src)

    nc.compile()
    return nc


def _round_f32r(x):
    """Round fp32 to f32r (11 explicit mantissa bits, round-to-nearest-even)."""
    xi = np.ascontiguousarray(np.asarray(x, np.float32)).view(np.uint32)
    k = 12
    lsb = (xi >> k) & 1
    half = np.uint32((1 << (k - 1)) - 1)
    out = ((xi + half + lsb) & np.uint32(~((1 << k) - 1) & 0xFFFFFFFF))
    return out.view(np.float32)


def _prep_host_inputs(random_vector, params):
    """Build per-core input maps (weights shared across cores)."""
    p = {k: [(np.asarray(w, np.float32), np.asarray(b, np.float32))
             for w, b in v] for k, v in params.items()}
    r = _round_f32r
    wgp = p["glob_pre"]; wgq = p["glob_post"]; wpj = p["proj"]
    wms = p["msg"]; wup = p["update"]
    shared = {
        "gp0": r(wgp[0][0]), "gp1": r(wgp[1][0]),
        "gq0": r(wgq[0][0]), "gq1": r(wgq[1][0]),
        "pj0x": r(wpj[0][0][0:F]), "pj0g": r(wpj[0][0][F:2 * F]),
        "pj1": r(wpj[1][0]), "pj2": r(wpj[2][0]),
        "ms0x": r(wms[0][0][0:F]), "ms0g": r(wms[0][0][F:2 * F]),
        "ms1": r(wms[1][0]), "ms2": r(wms[2][0]),
        "up0x": r(wup[0][0][0:F]), "up0a": r(wup[0][0][F:2 * F]),
        "up0g": r(wup[0][0][2 * F:2 * F + G]),
        "up1a": r(wup[1][0][0:2 * F]), "up1b": r(wup[1][0][2 * F:2 * F + G]),
        "up2": r(wup[2][0]),
        "bgp0": wgp[0][1].reshape(-1, 1), "bgp1": wgp[1][1].reshape(-1, 1),
        "bgq0": wgq[0][1].reshape(-1, 1), "bgq1": wgq[1][1].reshape(-1, 1),
        "bpj0": wpj[0][1].reshape(-1, 1), "bpj1": wpj[1][1].reshape(-1, 1),
        "bpj2a": wpj[2][1][0:F].reshape(-1, 1),
        "bpj2b": wpj[2][1][F:2 * F].reshape(-1, 1),
        "bms0": wms[0][1].reshape(-1, 1), "bms1": wms[1][1].reshape(-1, 1),
        "bms2": wms[2][1].reshape(-1, 1),
        "bup0a": wup[0][1][0:2 * F].reshape(-1, 1),
        "bup0b": wup[0][1][2 * F:2 * F + G].reshape(-1, 1),
        "bup1": wup[1][1].reshape(-1, 1), "bup2": wup[2][1].reshape(-1, 1),
    }
    rv = np.asarray(random_vector, np.float32).reshape(B, F)
    in_maps = []
    for c in range(NCORES):
        m = dict(shared)
        m["xin"] = r(np.ascontiguousarray(rv[c * BPC:(c + 1) * BPC].T))
        in_maps.append(m)
    return in_maps


def kernel(random_vector, params):
    if _cache["nc"] is None:
        _cache["nc"] = build_nc()
    nc = _cache["nc"]
    in_maps = _prep_host_inputs(random_vector, params)
    res = run_bass_kernel_spmd(nc, in_maps, core_ids=list(range(NCORES)))
    if res.exec_time_ns is not None:
        kernel.last_exec_time_ns = res.exec_time_ns
    outs = []
    for c in range(NCORES):
        xo = res.results[c]["xout"]              # [F, NODES*BPC]
        # free index = node*BPC + sample  ->  [sample, node, feat]
        outs.append(xo.reshape(F, NODES, BPC).transpose(2, 1, 0)
                    .reshape(BPC * NODES, F))
    return np.ascontiguousarray(np.concatenate(outs, axis=0))


kernel.last_exec_time_ns = None
